# revision 41
# baseline (speedup 1.0000x reference)
"""Batched Procrustes-alignment loss on 8 Trainium2 NeuronCores.

Data-parallel over batch (B=262144 -> 32768/core), laid out as [128
partitions, F=256] planes (one scalar per batch element per plane).

v2 pipeline (per core, per For_i iteration):
  DMA raw [P, 51*SUB] f32 sub-chunks; Act de-interleaves+casts to bf16
  component planes [P, 3*JF] (PC/TC after in-place centering). DVE fused-3
  bf16 j-trees give means; fused-3 in-place centered subtract.
  Squares via Act into small ping-pong scratch; folds -> P2/T2 bf16;
  Act sqrt -> sp2st2 (work slot W1); fused-2 j-tree -> pn/tn (PSUM);
  s = tn/(pn+eps) early; d2 = s^2*P2 + T2 seeded before the SVD tail.
  H phase: per r-group one fused-3 product TC_r*PC into work slot W2 and
  a fused-3 bf16 j-tree -> H[r*3+c] = H_{c,r} (f32).
  Closed-form 3x3 eigensolver on A = H^T H (A6 in PSUM, trig eigenvalues,
  eigvecs via cross-of-rows, u_i = H v_i * (-2s/sigma_i), u2 = cross/-2s);
  G' plane (r*3+c) = sum_i u_i[c] v_i[r] (-2s folded), assembled in bf16
  carved out of W1.
  e-phase (no stored O): E_acc[r] = sum_c PC_c * G'_{c,r} (bcast over j),
  then Et = E_acc * TC, folded into d2. dist = sqrt(max(d2,0)); j-tree
  sum; acc += dsum. Host sums [P,1] partials in float64.
"""
import numpy as np
import concourse.bass as bass
import concourse.mybir as mybir
import concourse.tile as tile
from concourse import bacc
from concourse.bass_utils import run_bass_kernel_spmd

AF = mybir.ActivationFunctionType
OP = mybir.AluOpType
AX = mybir.AxisListType
f32 = mybir.dt.float32
bf16 = mybir.dt.bfloat16

B, J, C = 262144, 17, 3
JC = J * C
NCORES = 8
BC = B // NCORES
P = 128
F = 256
JF = J * F
SUB = 64
NSUB = F // SUB
EPS = 1e-8
TINY = 1e-20

# engine assignment knobs ("v" = DVE vector, "g" = gpsimd Pool, "s" = scalar/Act)
KNOBS = dict(
    deint=["s"] * 8,          # per (tensor*NSUB + sub)
    meantree=["v", "v"],      # per tensor
    center=["v", "v"],        # per tensor
    p2fold=["v", "v", "v", "v"],   # P2+=sq1, P2+=sq2, T2+=sq1, T2+=sq2
    pntree="v",
    d2seed=["v", "v"],        # d2 = P2*s2, d2 += T2
    oprod=["v", "v", "v"],    # per r
    htree=["v", "v", "v"],    # per r
    eprod=["v", "v", "v"],    # per c
    eacc=["v", "v"],
    emul="v",
    wd2=["v", "v", "v"],      # three JF folds into d2
    d2max="v",
    dsum="v",
)


def _ap(t, off, dims):
    a = t[:]
    return bass.AP(a.tensor, a.offset + off, [a.ap[0]] + dims)


def _pl(t, off, n):
    return _ap(t, off, [[1, n]])


def build_nc(iters=1, knobs=None, stop=99, tap=None, unroll=4):
    kn = dict(KNOBS)
    if knobs:
        kn.update(knobs)
    TAP_SHAPES = dict(means=6 * F, PC=3 * JF, TC=3 * JF, P2T2=2 * JF,
                      pntn=2 * F, H=9 * F, G=9 * F, d2=JF)

    nc = bacc.Bacc("TRN2", target_bir_lowering=False)
    pred_d = nc.dram_tensor("pred", [BC, JC], f32, kind="ExternalInput")
    targ_d = nc.dram_tensor("target", [BC, JC], f32, kind="ExternalInput")
    out_d = nc.dram_tensor("partial", [P, 1], f32, kind="ExternalOutput")
    dbg_d = (nc.dram_tensor("dbg", [P, TAP_SHAPES[tap]],
                            f32 if tap in ("H", "pntn") else bf16,
                            kind="ExternalOutput") if tap else None)

    def E(key, i=None):
        v = kn[key] if i is None else kn[key][i]
        return {"v": nc.vector, "g": nc.gpsimd, "s": nc.scalar}[v]

    with tile.TileContext(nc) as tc:
        with (
            tc.tile_pool(name="persist", bufs=1) as persist,
            tc.tile_pool(name="rawp", bufs=1) as rawp,
            tc.tile_pool(name="pctc", bufs=1) as pctcp,
            tc.tile_pool(name="work", bufs=1) as work,
            tc.tile_pool(name="hp", bufs=1) as hp,
            tc.tile_pool(name="late", bufs=1) as late,
            tc.tile_pool(name="thinE", bufs=1) as thinE,
            tc.tile_pool(name="psth", bufs=1, space="PSUM") as psth,
        ):
            acc = persist.tile([P, F], f32, tag="acc", name="acc")
            b2p3 = persist.tile([P, 1], f32, tag="b2p3", name="b2p3")
            b4p3 = persist.tile([P, 1], f32, tag="b4p3", name="b4p3")
            nc.gpsimd.memset(acc[:], 0.0)
            nc.gpsimd.memset(b2p3[:], 2.0943951023931953)  # 2pi/3
            nc.gpsimd.memset(b4p3[:], 1.0471975511965976)  # pi/3
            c_eps = persist.tile([P, 1], f32, tag="ceps", name="c_eps")
            c_tiny = persist.tile([P, 1], f32, tag="ctny", name="c_tiny")
            c_e10 = persist.tile([P, 1], f32, tag="ce10", name="c_e10")
            nc.gpsimd.memset(c_eps[:], EPS)
            nc.gpsimd.memset(c_tiny[:], TINY)
            nc.gpsimd.memset(c_e10[:], 1e-10)

            def thinE_t():
                return thinE.tile([P, F], f32, tag="te", name="te", bufs=9)

            def named(tg):
                return thinE.tile([P, F], f32, tag="An", name=tg, bufs=6)

            _ps = {"n": 0, "banks": []}

            def psum_t(tg):
                i = _ps["n"]
                _ps["n"] += 1
                assert i < 16
                if i % 2 == 0:
                    _ps["banks"].append(
                        psth.tile([P, 2 * F], f32, tag=f"pb{i // 2}",
                                  name=f"pb{i // 2}"))
                blk = _ps["banks"][i // 2]
                off = (i % 2) * F

                class _T:
                    def __getitem__(self, _):
                        return _pl(blk, off, F)
                return _T()

            def body():
                _ps["n"] = 0
                _ps["banks"] = []
                # --------- persistent-ish tiles for this iteration
                PC = pctcp.tile([P, 3 * JF], bf16, tag="PC", name="PC")
                TC = pctcp.tile([P, 3 * JF], bf16, tag="TC", name="TC")
                means = pctcp.tile([P, 6 * F], bf16, tag="mn", name="means")
                ht = hp.tile([P, 3 * 8 * F], bf16, tag="ht", name="ht")
                H = hp.tile([P, 9 * F], bf16, tag="H", name="H")
                d2 = late.tile([P, JF], bf16, tag="d2", name="d2")
                P2T2 = late.tile([P, 2 * JF], bf16, tag="p2", name="P2T2")
                Gp = late.tile([P, 9 * F], bf16, tag="G", name="Gp")

                def g3(t, off, inner=F):
                    return _ap(t, off, [[JF, 3], [F, J], [1, inner]])

                # --------- load + Act de-interleave/cast + mean + center
                for ti, (dram, ctr) in enumerate(((pred_d, PC), (targ_d, TC))):
                    for s in range(NSUB):
                        raw = rawp.tile([P, JC * SUB], f32, tag="raw",
                                        name="raw", bufs=2)
                        off = (s * SUB) * JC
                        nc.sync.dma_start(
                            raw[:], bass.AP(dram[:].tensor, off,
                                            [[F * JC, P], [1, JC * SUB]]))
                        # ctr[c][j][s*SUB+u] = raw[u*JC + j*3 + c]
                        de = E("deint", ti * NSUB + s)
                        dd_ = (_ap(ctr, s * SUB, [[JF, 3], [F, J], [1, SUB]]),
                               _ap(raw, 0, [[1, 3], [3, J], [JC, SUB]]))
                        if de is nc.scalar:
                            de.activation(dd_[0], dd_[1], AF.Copy)
                        else:
                            de.tensor_copy(dd_[0], dd_[1])
                    mn = _ap(means, ti * 3 * F, [[F, 3], [1, F]])
                    et = E("meantree", ti)
                    et.tensor_tensor(
                        _ap(ht, 0, [[8 * F, 3], [F, 8], [1, F]]),
                        _ap(ctr, 0, [[JF, 3], [F, 8], [1, F]]),
                        _ap(ctr, 8 * F, [[JF, 3], [F, 8], [1, F]]), OP.add)
                    et.tensor_tensor(
                        _ap(ht, 0, [[8 * F, 3], [F, 4], [1, F]]),
                        _ap(ht, 0, [[8 * F, 3], [F, 4], [1, F]]),
                        _ap(ht, 4 * F, [[8 * F, 3], [F, 4], [1, F]]), OP.add)
                    et.tensor_tensor(
                        _ap(ht, 0, [[8 * F, 3], [F, 2], [1, F]]),
                        _ap(ht, 0, [[8 * F, 3], [F, 2], [1, F]]),
                        _ap(ht, 2 * F, [[8 * F, 3], [F, 2], [1, F]]), OP.add)
                    et.tensor_tensor(
                        _ap(ht, 0, [[8 * F, 3], [1, F]]),
                        _ap(ht, 0, [[8 * F, 3], [1, F]]),
                        _ap(ht, F, [[8 * F, 3], [1, F]]), OP.add)
                    et.tensor_tensor(
                        mn,
                        _ap(ht, 0, [[8 * F, 3], [1, F]]),
                        _ap(ctr, 16 * F, [[JF, 3], [1, F]]), OP.add)
                    nc.scalar.mul(mn, mn, 1.0 / J)
                    E("center", ti).tensor_tensor(
                        g3(ctr, 0), g3(ctr, 0),
                        _ap(means, ti * 3 * F, [[F, 3], [0, J], [1, F]]),
                        OP.subtract)

                if tap == "means":
                    nc.sync.dma_start(dbg_d[:], means[:])
                if tap == "PC":
                    nc.sync.dma_start(dbg_d[:], PC[:])
                if tap == "TC":
                    nc.sync.dma_start(dbg_d[:], TC[:])
                if stop <= 0:
                    return

                # --------- squares -> P2/T2 (Act into scratch, DVE folds)
                P2 = _pl(P2T2, 0, JF)
                T2 = _pl(P2T2, JF, JF)
                # W1a hosts: squares scratch (3rd JF plane), then sp2st2
                # (planes 0-1) for the pn/tn tree
                W1 = work.tile([P, 3 * JF], bf16, tag="W1", name="W1a")
                sqh = _pl(W1, 2 * JF, JF)
                sqd = d2[:]              # d2 free until seed
                for ti, (ctr, dst) in enumerate(((PC, P2), (TC, T2))):
                    nc.scalar.activation(dst, _pl(ctr, 0, JF), AF.Square)
                    nc.scalar.activation(sqh, _pl(ctr, JF, JF), AF.Square)
                    nc.scalar.activation(sqd, _pl(ctr, 2 * JF, JF), AF.Square)
                    E("p2fold", ti * 2).tensor_tensor(dst, dst, sqh, OP.add)
                    E("p2fold", ti * 2 + 1).tensor_tensor(dst, dst, sqd, OP.add)

                # --------- H phase + A6 interleaved
                # Op plane (r*3+c) = TC_r * PC_c ; H plane (r*3+c) = H_{c,r}
                W2 = work.tile([P, 3 * JF], bf16, tag="W2", name="W2a")

                def h_group(r):
                    E("oprod", r).tensor_tensor(
                        g3(W2, 0),
                        _ap(TC, r * JF, [[0, 3], [F, J], [1, F]]),
                        g3(PC, 0), OP.mult)
                    et = E("htree", r)
                    et.tensor_tensor(
                        _ap(ht, 0, [[8 * F, 3], [F, 8], [1, F]]),
                        _ap(W2, 0, [[JF, 3], [F, 8], [1, F]]),
                        _ap(W2, 8 * F, [[JF, 3], [F, 8], [1, F]]), OP.add)
                    et.tensor_tensor(
                        _ap(ht, 0, [[8 * F, 3], [F, 4], [1, F]]),
                        _ap(ht, 0, [[8 * F, 3], [F, 4], [1, F]]),
                        _ap(ht, 4 * F, [[8 * F, 3], [F, 4], [1, F]]), OP.add)
                    et.tensor_tensor(
                        _ap(ht, 0, [[8 * F, 3], [F, 2], [1, F]]),
                        _ap(ht, 0, [[8 * F, 3], [F, 2], [1, F]]),
                        _ap(ht, 2 * F, [[8 * F, 3], [F, 2], [1, F]]), OP.add)
                    et.tensor_tensor(
                        _ap(ht, 0, [[8 * F, 3], [1, F]]),
                        _ap(ht, 0, [[8 * F, 3], [1, F]]),
                        _ap(ht, F, [[8 * F, 3], [1, F]]), OP.add)
                    et.tensor_tensor(
                        _ap(H, r * 3 * F, [[F, 3], [1, F]]),
                        _ap(ht, 0, [[8 * F, 3], [1, F]]),
                        _ap(W2, 16 * F, [[JF, 3], [1, F]]), OP.add)

                def Hp(a, cc):
                    # H_{cc,a} (pred comp cc, targ comp a) = plane (a*3+cc)
                    return _pl(H, (a * 3 + cc) * F, F)


                a6s = hp.tile([P, 4 * F], bf16, tag="a6s", name="a6s")
                # contiguous A block: [a00, a11, a22 | a01, a02, a12]
                Ad = hp.tile([P, 6 * F], f32, tag="Ad", name="Ad")
                A_IDX = {(0, 0): 0, (1, 1): 1, (2, 2): 2,
                         (0, 1): 3, (0, 2): 4, (1, 2): 5}

                def _w(ap):
                    class _T:
                        def __getitem__(self, _):
                            return ap
                    return _T()

                def a6_entry(a, b):
                    # one fused-3 bf16 product over cc, then fold
                    nc.vector.tensor_tensor(
                        _ap(a6s, 0, [[F, 3], [1, F]]),
                        _ap(H, a * 3 * F, [[F, 3], [1, F]]),
                        _ap(H, b * 3 * F, [[F, 3], [1, F]]), OP.mult)
                    nc.vector.tensor_tensor(
                        _pl(a6s, 3 * F, F), _pl(a6s, 0, F), _pl(a6s, F, F),
                        OP.add)
                    nc.vector.tensor_tensor(
                        _pl(Ad, A_IDX[(a, b)] * F, F),
                        _pl(a6s, 3 * F, F), _pl(a6s, 2 * F, F), OP.add)

                h_group(0)
                h_group(1)
                a6_entry(0, 0)
                a6_entry(0, 1)
                a6_entry(1, 1)
                h_group(2)
                a6_entry(0, 2)
                a6_entry(1, 2)
                a6_entry(2, 2)

                # --------- sqrt -> sp2st2 (W1 planes 0-1); pn/tn tree; s
                sp2st2 = _pl(W1, 0, 2 * JF)
                nc.scalar.activation(sp2st2, P2T2[:], AF.Sqrt)
                pntn = psth.tile([P, 2 * F], f32, tag="pntn", name="pntn")
                et = E("pntree")
                et.tensor_tensor(
                    _ap(ht, 0, [[8 * F, 2], [F, 8], [1, F]]),
                    _ap(W1, 0, [[JF, 2], [F, 8], [1, F]]),
                    _ap(W1, 8 * F, [[JF, 2], [F, 8], [1, F]]), OP.add)
                et.tensor_tensor(
                    _ap(ht, 0, [[8 * F, 2], [F, 4], [1, F]]),
                    _ap(ht, 0, [[8 * F, 2], [F, 4], [1, F]]),
                    _ap(ht, 4 * F, [[8 * F, 2], [F, 4], [1, F]]), OP.add)
                et.tensor_tensor(
                    _ap(ht, 0, [[8 * F, 2], [F, 2], [1, F]]),
                    _ap(ht, 0, [[8 * F, 2], [F, 2], [1, F]]),
                    _ap(ht, 2 * F, [[8 * F, 2], [F, 2], [1, F]]), OP.add)
                et.tensor_tensor(
                    _ap(ht, 0, [[8 * F, 2], [1, F]]),
                    _ap(ht, 0, [[8 * F, 2], [1, F]]),
                    _ap(ht, F, [[8 * F, 2], [1, F]]), OP.add)
                et.tensor_tensor(
                    _ap(pntn, 0, [[F, 2], [1, F]]),
                    _ap(ht, 0, [[8 * F, 2], [1, F]]),
                    _ap(W1, 16 * F, [[JF, 2], [1, F]]), OP.add)
                pn = _pl(pntn, 0, F)
                tn = _pl(pntn, F, F)
                if tap == "P2T2":
                    nc.sync.dma_start(dbg_d[:], P2T2[:])
                if tap == "pntn":
                    pncp = late.tile([P, 2 * F], f32, tag="pncp", name="pncp")
                    nc.vector.tensor_copy(pncp[:], pntn[:])
                    nc.sync.dma_start(dbg_d[:], pncp[:])

                # s = tn/(pn+eps); s2 bf16 (SBUF); seed d2 = s^2*P2 + T2
                sS = psum_t("sS")
                st_ = thinE_t()
                nc.scalar.add(st_[:], pn, c_eps[:])
                nc.vector.reciprocal_approx_fast(st_[:], st_[:])
                nc.vector.tensor_tensor(sS[:], st_[:], tn, OP.mult)
                s2b = late.tile([P, F], bf16, tag="s2b", name="s2b")
                nc.scalar.activation(s2b[:], sS[:], AF.Square)
                E("d2seed", 0).tensor_tensor(
                    d2[:], P2, _ap(s2b, 0, [[0, J], [1, F]]), OP.mult)
                E("d2seed", 1).tensor_tensor(d2[:], d2[:], T2, OP.add)

                if stop <= 2:
                    return

                a00 = _w(_pl(Ad, 0, F))
                a11 = _w(_pl(Ad, F, F))
                a22 = _w(_pl(Ad, 2 * F, F))
                a01 = _w(_pl(Ad, 3 * F, F))
                a02 = _w(_pl(Ad, 4 * F, F))
                a12 = _w(_pl(Ad, 5 * F, F))

                # --------- eigenvalues (closed form, f32)
                q3 = thinE_t()
                nc.vector.tensor_tensor(q3[:], a00[:], a11[:], OP.add)
                nc.vector.tensor_tensor(q3[:], q3[:], a22[:], OP.add)
                mb = hp.tile([P, 3 * F], f32, tag="mb", name="mb")
                nc.scalar.activation(mb[:], _pl(Ad, 3 * F, 3 * F), AF.Square)
                m01 = _w(_pl(mb, 0, F))
                m02 = _w(_pl(mb, F, F))
                m12 = _w(_pl(mb, 2 * F, F))
                g0, g1, g2 = named("g0"), named("g1"), named("g2")
                nc.vector.tensor_tensor(g0[:], a01[:], a12[:], OP.mult)
                nc.vector.tensor_tensor(g1[:], a01[:], a02[:], OP.mult)
                nc.vector.tensor_tensor(g2[:], a02[:], a12[:], OP.mult)
                p1 = thinE_t()
                nc.vector.tensor_tensor(p1[:], m01[:], m02[:], OP.add)
                nc.vector.tensor_tensor(p1[:], p1[:], m12[:], OP.add)
                q = named("q")
                nc.scalar.mul(q[:], q3[:], 1.0 / 3)
                bbk = hp.tile([P, 3 * F], f32, tag="bb", name="bbk")
                nc.vector.tensor_tensor(
                    bbk[:], _pl(Ad, 0, 3 * F),
                    _ap(q, 0, [[0, 3], [1, F]]), OP.subtract)
                b00 = _w(_pl(bbk, 0, F))
                b11 = _w(_pl(bbk, F, F))
                b22 = _w(_pl(bbk, 2 * F, F))
                p2s = thinE_t()
                nc.vector.tensor_tensor(p2s[:], b00[:], b00[:], OP.mult)
                tb = thinE_t()
                nc.vector.tensor_tensor(tb[:], b11[:], b11[:], OP.mult)
                nc.vector.tensor_tensor(p2s[:], p2s[:], tb[:], OP.add)
                nc.vector.tensor_tensor(tb[:], b22[:], b22[:], OP.mult)
                nc.vector.tensor_tensor(p2s[:], p2s[:], tb[:], OP.add)
                nc.vector.scalar_tensor_tensor(
                    p2s[:], p1[:], 2.0, p2s[:], OP.mult, OP.add)
                pA = named("pA")
                nc.scalar.activation(pA[:], p2s[:], AF.Sqrt, scale=1.0 / 6)
                c0 = thinE_t()
                nc.vector.tensor_tensor(c0[:], b11[:], b22[:], OP.mult)
                nc.vector.tensor_tensor(c0[:], c0[:], m12[:], OP.subtract)
                c1 = thinE_t()
                nc.vector.tensor_tensor(c1[:], a01[:], b22[:], OP.mult)
                nc.vector.tensor_tensor(c1[:], c1[:], g2[:], OP.subtract)
                c2 = thinE_t()
                nc.vector.tensor_tensor(c2[:], b11[:], a02[:], OP.mult)
                nc.vector.tensor_tensor(c2[:], g0[:], c2[:], OP.subtract)
                detB = thinE_t()
                nc.vector.tensor_tensor(detB[:], b00[:], c0[:], OP.mult)
                tdb = thinE_t()
                nc.vector.tensor_tensor(tdb[:], a01[:], c1[:], OP.mult)
                nc.vector.tensor_tensor(detB[:], detB[:], tdb[:], OP.subtract)
                nc.vector.tensor_tensor(tdb[:], a02[:], c2[:], OP.mult)
                nc.vector.tensor_tensor(detB[:], detB[:], tdb[:], OP.add)
                pinv = thinE_t()
                nc.scalar.add(pinv[:], pA[:], c_tiny[:])
                nc.vector.reciprocal_approx_fast(pinv[:], pinv[:])
                p3 = thinE_t()
                nc.vector.tensor_tensor(p3[:], pinv[:], pinv[:], OP.mult)
                nc.vector.tensor_tensor(p3[:], p3[:], pinv[:], OP.mult)
                rc = thinE_t()
                nc.vector.tensor_tensor(rc[:], detB[:], p3[:], OP.mult)
                nc.vector.tensor_scalar(rc[:], rc[:], 0.5, 1.0, OP.mult, OP.min)
                nc.vector.tensor_scalar_max(rc[:], rc[:], -1.0)
                rr = thinE_t()
                nc.vector.tensor_tensor(rr[:], rc[:], rc[:], OP.mult)
                wA = thinE_t()
                nc.scalar.activation(wA[:], rr[:], AF.Sqrt, bias=1.0, scale=-1.0)
                rat = thinE_t()
                nc.scalar.add(rat[:], wA[:], c_e10[:])
                nc.vector.reciprocal_approx_fast(rat[:], rat[:])
                nc.vector.tensor_tensor(rat[:], rc[:], rat[:], OP.mult)
                a1 = thinE_t()
                nc.vector.tensor_scalar(a1[:], rat[:], 1.0, -1.0, OP.min, OP.max)
                rat2 = thinE_t()
                nc.vector.tensor_tensor(rat2[:], rat[:], rat[:], OP.mult)
                rinv = thinE_t()
                nc.scalar.add(rinv[:], rat2[:], c_tiny[:])
                nc.vector.reciprocal_approx_fast(rinv[:], rinv[:])
                nc.vector.tensor_tensor(rinv[:], rat[:], rinv[:], OP.mult)
                nc.vector.tensor_scalar(rinv[:], rinv[:], 1.0, -1.0, OP.min, OP.max)
                sg = thinE_t()
                nc.vector.tensor_scalar(sg[:], rat[:], 1e10, 1.0, OP.mult, OP.min)
                nc.vector.tensor_scalar_max(sg[:], sg[:], -1.0)
                at1 = thinE_t()
                nc.scalar.activation(at1[:], a1[:], AF.Arctan)
                at2 = thinE_t()
                nc.scalar.activation(at2[:], rinv[:], AF.Arctan)
                atb = thinE_t()
                nc.vector.scalar_tensor_tensor(
                    atb[:], sg[:], 1.5707963267948966, at2[:],
                    OP.mult, OP.subtract)
                m_ = thinE_t()
                nc.vector.tensor_scalar_add(m_[:], rat2[:], -1.0)
                nc.vector.tensor_scalar(m_[:], m_[:], 1e10, 1.0, OP.mult, OP.min)
                nc.vector.tensor_scalar_max(m_[:], m_[:], 0.0)
                atn = thinE_t()
                nc.vector.tensor_tensor(atn[:], atb[:], at1[:], OP.subtract)
                nc.vector.tensor_tensor(atn[:], atn[:], m_[:], OP.mult)
                nc.vector.tensor_tensor(atn[:], atn[:], at1[:], OP.add)
                cs1 = psum_t("cs1")
                nc.scalar.activation(cs1[:], atn[:], AF.Sin,
                                     bias=b2p3[:], scale=-1.0 / 3)
                cs2 = psum_t("cs2")
                nc.scalar.activation(cs2[:], atn[:], AF.Sin,
                                     bias=b4p3[:], scale=-1.0 / 3)
                lam0, lam1 = psum_t("lam0"), psum_t("lam1")
                tp = thinE_t()
                nc.vector.tensor_tensor(tp[:], pA[:], cs1[:], OP.mult)
                nc.vector.scalar_tensor_tensor(
                    lam0[:], tp[:], 2.0, q[:], OP.mult, OP.add)
                lam2 = thinE_t()
                nc.vector.tensor_tensor(tp[:], pA[:], cs2[:], OP.mult)
                nc.vector.scalar_tensor_tensor(
                    lam2[:], tp[:], -2.0, q[:], OP.mult, OP.add)
                nc.vector.scalar_tensor_tensor(
                    lam1[:], q[:], 3.0, lam0[:], OP.mult, OP.subtract)
                nc.vector.tensor_tensor(lam1[:], lam1[:], lam2[:], OP.subtract)

                # --------- W1 carve for bf16 tail (sp2st2 dead after
                # pntree). layout: Hb 9F | vb 9F | ub 6F | u2t 3F | gt 3F |
                # gt2 3F | rsb 2F | invsb F
                W1b = work.tile([P, 3 * JF], bf16, tag="W1", name="W1b")
                invsb = _pl(W1b, 35 * F, F)

                def vbp(i, k):
                    return _pl(W1b, (9 + i * 3 + k) * F, F)

                # --------- eigenvectors v0, v1 (f32 transient -> bf16 vb)
                def eigvec(lam, vbi):
                    vx = thinE_t()
                    vy = thinE_t()
                    vz = thinE_t()
                    b0 = thinE_t()
                    nc.vector.tensor_tensor(b0[:], a00[:], lam[:], OP.subtract)
                    b1 = thinE_t()
                    nc.vector.tensor_tensor(b1[:], a11[:], lam[:], OP.subtract)
                    nc.vector.tensor_tensor(vx[:], a02[:], b1[:], OP.mult)
                    nc.vector.tensor_tensor(vx[:], g0[:], vx[:], OP.subtract)
                    nc.vector.tensor_tensor(vy[:], b0[:], a12[:], OP.mult)
                    nc.vector.tensor_tensor(vy[:], g1[:], vy[:], OP.subtract)
                    nc.vector.tensor_tensor(vz[:], b0[:], b1[:], OP.mult)
                    nc.vector.tensor_tensor(vz[:], vz[:], m01[:], OP.subtract)
                    n2 = thinE_t()
                    nc.vector.tensor_tensor(n2[:], vx[:], vx[:], OP.mult)
                    t2_ = thinE_t()
                    nc.vector.tensor_tensor(t2_[:], vy[:], vy[:], OP.mult)
                    nc.vector.tensor_tensor(n2[:], n2[:], t2_[:], OP.add)
                    nc.vector.tensor_tensor(t2_[:], vz[:], vz[:], OP.mult)
                    nc.vector.tensor_tensor(n2[:], n2[:], t2_[:], OP.add)
                    ns = thinE_t()
                    nc.scalar.activation(ns[:], n2[:], AF.Sqrt)
                    nc.scalar.add(ns[:], ns[:], c_tiny[:])
                    nc.vector.reciprocal_approx_fast(ns[:], ns[:])
                    nc.vector.tensor_tensor(vbp(vbi, 0), vx[:], ns[:], OP.mult)
                    nc.vector.tensor_tensor(vbp(vbi, 1), vy[:], ns[:], OP.mult)
                    nc.vector.tensor_tensor(vbp(vbi, 2), vz[:], ns[:], OP.mult)

                eigvec(lam0, 0)
                eigvec(lam1, 1)

                # v2 = v0 x v1 (bf16)
                cr = ((1, 2), (2, 0), (0, 1))
                for r_ in range(3):
                    i1, i2 = cr[r_]
                    t1b = _pl(W1b, 27 * F, F)
                    t2b = _pl(W1b, 28 * F, F)
                    nc.vector.tensor_tensor(t1b, vbp(0, i1), vbp(1, i2),
                                            OP.mult)
                    nc.vector.tensor_tensor(t2b, vbp(0, i2), vbp(1, i1),
                                            OP.mult)
                    nc.vector.tensor_tensor(vbp(2, r_), t1b, t2b, OP.subtract)

                # --------- rsig_i = -2s/sigma_i (bf16 into rsb)
                for i, lam in enumerate((lam0, lam1)):
                    rl = thinE_t()
                    nc.scalar.activation(rl[:], lam[:], AF.Relu)
                    sg_ = thinE_t()
                    nc.scalar.activation(sg_[:], rl[:], AF.Sqrt)
                    nc.scalar.add(sg_[:], sg_[:], c_tiny[:])
                    nc.vector.reciprocal_approx_fast(sg_[:], sg_[:])
                    nc.vector.scalar_tensor_tensor(
                        _pl(W1b, (33 + i) * F, F), sg_[:], -2.0, sS[:],
                        OP.mult, OP.mult)
                iv_ = thinE_t()
                nc.scalar.add(iv_[:], sS[:], c_tiny[:])
                nc.vector.reciprocal_approx_fast(iv_[:], iv_[:])
                nc.scalar.mul(invsb, iv_[:], -0.5)

                def HCg(k):
                    # planes (k*3 + r) = H_{r,k}, r=0..2
                    return _ap(H, k * 3 * F, [[F, 3], [1, F]])

                def vbc(i, k):
                    return _ap(W1b, (9 + i * 3 + k) * F, [[0, 3], [1, F]])

                # u_i[r] = sum_k H_{r,k} (v_i)_k, both i at once
                ubp = _ap(W1b, 18 * F, [[3 * F, 2], [F, 3], [1, F]])

                def HCg2(k):
                    return _ap(H, k * 3 * F, [[0, 2], [F, 3], [1, F]])

                def vbc2(k):
                    return _ap(W1b, (9 + k) * F, [[3 * F, 2], [0, 3], [1, F]])

                gp2 = _ap(W1b, 27 * F, [[3 * F, 2], [F, 3], [1, F]])
                nc.vector.tensor_tensor(ubp, HCg2(0), vbc2(0), OP.mult)
                nc.vector.tensor_tensor(gp2, HCg2(1), vbc2(1), OP.mult)
                nc.vector.tensor_tensor(ubp, ubp, gp2, OP.add)
                nc.vector.tensor_tensor(gp2, HCg2(2), vbc2(2), OP.mult)
                nc.vector.tensor_tensor(ubp, ubp, gp2, OP.add)
                nc.vector.tensor_tensor(
                    ubp, ubp,
                    _ap(W1b, 33 * F, [[F, 2], [0, 3], [1, F]]), OP.mult)

                def up(ui, r_):
                    return _pl(W1b, (18 + ui * 3 + r_) * F, F)

                # u2 = cross(u0, u1) * (-0.5/s)
                for r_ in range(3):
                    i1, i2 = cr[r_]
                    t1b = _pl(W1b, 27 * F, F)
                    t2b = _pl(W1b, 28 * F, F)
                    nc.vector.tensor_tensor(t1b, up(0, i1), up(1, i2), OP.mult)
                    nc.vector.tensor_tensor(t2b, up(0, i2), up(1, i1), OP.mult)
                    nc.vector.tensor_tensor(t1b, t1b, t2b, OP.subtract)
                    nc.vector.tensor_tensor(
                        _pl(W1b, (24 + r_) * F, F), t1b, invsb, OP.mult)

                # --------- G' plane (r*3+c) = sum_i u_i[c] * (v_i)_r
                def ug(i):
                    base = (18 + i * 3) * F if i < 2 else 24 * F
                    return _ap(W1b, base, [[F, 3], [1, F]])

                # G' plane (r*3+c) = sum_i u_i[c] * v_r[i]  (V^T quirk of
                # the reference: R = Vh @ Ut). One [P,9F] op per i-term;
                # scratch = W1b planes 27F..36F (gt/gt2/rsb/invsb, all dead).
                Gall = _ap(Gp, 0, [[3 * F, 3], [F, 3], [1, F]])
                Wsc = _ap(W1b, 27 * F, [[3 * F, 3], [F, 3], [1, F]])

                def ugb(i):
                    return _ap(W1b, (18 + 3 * i) * F, [[0, 3], [F, 3], [1, F]])

                def vrb(i):
                    return _ap(W1b, (9 + i) * F, [[3 * F, 3], [0, 3], [1, F]])

                nc.vector.tensor_tensor(Gall, ugb(0), vrb(0), OP.mult)
                nc.vector.tensor_tensor(Wsc, ugb(1), vrb(1), OP.mult)
                nc.vector.tensor_tensor(
                    _pl(Gp, 0, 9 * F), _pl(Gp, 0, 9 * F),
                    _pl(W1b, 27 * F, 9 * F), OP.add)
                nc.vector.tensor_tensor(Wsc, ugb(2), vrb(2), OP.mult)
                nc.vector.tensor_tensor(
                    _pl(Gp, 0, 9 * F), _pl(Gp, 0, 9 * F),
                    _pl(W1b, 27 * F, 9 * F), OP.add)

                if tap == "G":
                    nc.sync.dma_start(dbg_d[:], Gp[:])
                if stop <= 3:
                    return

                # --------- e-phase: E_acc[r] = sum_c PC_c * G'_{c,r}
                W2b = work.tile([P, 3 * JF], bf16, tag="W2", name="W2b")
                Ea = g3(W2b, 0)
                W1c = work.tile([P, 3 * JF], bf16, tag="W1", name="W1c")
                Et = g3(W1c, 0)

                def gpc(c):
                    # G' planes (r*3+c) for r=0..2: offset c*F, stride 3F
                    return _ap(Gp, c * F, [[3 * F, 3], [0, J], [1, F]])

                E("eprod", 0).tensor_tensor(
                    Ea, _ap(PC, 0, [[0, 3], [F, J], [1, F]]), gpc(0), OP.mult)
                E("eprod", 1).tensor_tensor(
                    Et, _ap(PC, JF, [[0, 3], [F, J], [1, F]]), gpc(1), OP.mult)
                E("eacc", 0).tensor_tensor(Ea, Ea, Et, OP.add)
                E("eprod", 2).tensor_tensor(
                    Et, _ap(PC, 2 * JF, [[0, 3], [F, J], [1, F]]), gpc(2),
                    OP.mult)
                E("eacc", 1).tensor_tensor(Ea, Ea, Et, OP.add)
                # Et = E_acc * TC (aligned r-planes); fold into d2
                E("emul").tensor_tensor(Et, Ea, g3(TC, 0), OP.mult)
                for c in range(3):
                    E("wd2", c).tensor_tensor(
                        d2[:], d2[:], _pl(W1c, c * JF, JF), OP.add)

                if tap == "d2":
                    nc.sync.dma_start(dbg_d[:], d2[:])
                # --------- dist = sqrt(max(d2,0)); j-tree; accumulate
                E("d2max").tensor_scalar_max(d2[:], d2[:], 0.0)
                dr = _pl(W1c, 0, JF)     # Et dead after wd2
                nc.scalar.activation(dr, d2[:], AF.Sqrt)
                dh = Gp  # dist-tree scratch aliases G (dead after e-prods)
                et = E("dsum")
                et.tensor_tensor(
                    _ap(dh, 0, [[F, 8], [1, F]]),
                    _ap(W1c, 0, [[F, 8], [1, F]]),
                    _ap(W1c, 8 * F, [[F, 8], [1, F]]), OP.add)
                et.tensor_tensor(
                    _ap(dh, 0, [[F, 4], [1, F]]),
                    _ap(dh, 0, [[F, 4], [1, F]]),
                    _ap(dh, 4 * F, [[F, 4], [1, F]]), OP.add)
                et.tensor_tensor(
                    _ap(dh, 0, [[F, 2], [1, F]]),
                    _ap(dh, 0, [[F, 2], [1, F]]),
                    _ap(dh, 2 * F, [[F, 2], [1, F]]), OP.add)
                et.tensor_tensor(
                    _pl(dh, 0, F), _pl(dh, 0, F), _pl(dh, F, F), OP.add)
                et.tensor_tensor(
                    _pl(dh, 0, F), _pl(dh, 0, F), _pl(W1c, 16 * F, F), OP.add)
                nc.vector.tensor_tensor(acc[:], acc[:], _pl(dh, 0, F), OP.add)

            # The all-engine barrier For_i inserts per trip serializes
            # iterations; unrolling several bodies per trip amortizes it and
            # lets body k+1's front overlap body k's tail via tag-ring deps.
            u = unroll
            while u > 1 and iters % u:
                u -= 1
            if iters == u or iters == 1:
                for _ in range(max(iters, 1)):
                    body()
            else:
                with tc.For_i(0, iters // u, 1):
                    for _ in range(u):
                        body()

            accs = persist.tile([P, 1], f32, tag="accs", name="accs")
            nc.vector.tensor_reduce(accs[:], acc[:], axis=AX.X, op=OP.add)
            nc.sync.dma_start(out_d[:], accs[:])

    nc.compile()
    return nc


def build_tapped(tap):
    nc = build_nc(iters=1, tap=tap)
    return nc, (lambda x: x)


_nc_cache = None


def get_nc():
    global _nc_cache
    if _nc_cache is None:
        _nc_cache = build_nc()
    return _nc_cache


def run(nc, pred, target, trace=False, **kw):
    pred2 = np.ascontiguousarray(np.asarray(pred), np.float32).reshape(B, JC)
    targ2 = np.ascontiguousarray(np.asarray(target), np.float32).reshape(B, JC)
    in_maps = [
        {"pred": pred2[c * BC:(c + 1) * BC], "target": targ2[c * BC:(c + 1) * BC]}
        for c in range(NCORES)
    ]
    res = run_bass_kernel_spmd(nc, in_maps, list(range(NCORES)), trace=trace, **kw)
    total = sum(r["partial"].astype(np.float64).sum() for r in res.results)
    loss = np.float32(total / (B * J))
    return loss, res


def kernel(pred, target):
    loss, _ = run(get_nc(), pred, target)
    return loss
